# revision 5
# baseline (speedup 1.0000x reference)
"""Trainium2 Bass kernel for nn_CCALoss (CLIP loss + concept BCE + Jaccard-softmax KL).

Sharding: data-parallel over batch rows. Each of the 8 cores receives B/8 = 64
rows of every [B, *] tensor plus the full transposed concept matrix (the
"all-gather" is done host-side since the kernel receives full inputs anyway).

Algebra (w = (mc == 1) is binary {0,1}):
    inter[i,j] = w_i . w_j                                  -> PE DoubleRow matmul
    union[i,j] = s_i + s_j - inter[i,j]
      s_j - inter[i,j] = sum_c (1 - w_i[c]) w_j[c]          -> PE DoubleRow matmul
      s_i                                                   -> rank-1 bf16 matmul
                                                               (s row x ones row)
so psum_u holds union directly; DVE does urec = 1/union, sim = inter * urec.
KL row terms reduce to d/(T*se) - ln se + ln scis with d = sum_j e_j*(sim-T*cis),
e = exp(sim/T); per-partition partial sums ship to the host, which does the
final ln/divide arithmetic in float64.

BCE: softplus has no ACT table here, so softplus(x) = ln(1 + exp(x)) in two ACT
ops (|x| <= ~5 so the unstable form is exact enough). Host masking bakes
mc == -1 entries to -30 (softplus ~= 0) and ships clst = cls * (mc == 1), so the
device only needs exp, ln+accum, and one reduce.

CLIP: host pre-rolls each logits row so the label diagonal sits in column 0;
LSE is roll-invariant. One [128,512] exp+accum per core gives the row sums, a
1-column copy ships the diagonal.

Layouts: [64, 512] row-major work is reshaped to a "split" [128, 256] layout
(row i cols 0:256 -> partition i; cols 256:512 -> partition 64+i). DoubleRow
matmuls contract all 256 concepts in one instruction (two 128-channel planes).

DMA plan: wpack (fp8 weights + s row) on SP's HWDGE queue first; fpack (rolled
logits, bf16) second on SP; bpack (cis | masked cls | clst, bf16) through the
Pool SWDGE path, which does not contend for the shared HWDGE unit. The out DMA
is a [128, 8] f32 stats tile.

Sync: raw Bass, standalone wait_ge only for cross-engine deps (same-engine
ordering is program order; eliding self-waits keeps the sequencers off the
critical path).
"""

from contextlib import ExitStack

import numpy as np

import concourse.bass as bass
import concourse.mybir as mybir
from concourse.bass_utils import run_bass_kernel_spmd

AF = mybir.ActivationFunctionType
ALU = mybir.AluOpType
AX = mybir.AxisListType

F32 = mybir.dt.float32
BF16 = mybir.dt.bfloat16
F8 = mybir.dt.float8e4
F8NP = mybir.dt.np(F8)
BF16NP = mybir.dt.np(BF16)

B = 512  # batch
C = 256  # concepts
M = 8  # cores
R = B // M  # rows per core = 64
P = 128
H = 256  # split-layout free size (B/2)
HC = 128  # split-layout free size for [R, C] tensors (C/2)
TEMP = 0.07
CONCEPT_WEIGHT = 0.5
CONCEPT_SIM_WEIGHT = 0.3

# wpack fp8 cols: comp_dr(128) | wT_h0_dr(512) | wT_h1_dr(512) | ws_dr(128) |
#   s row as bf16 bytes (256B = 128 cols x 2, partition 0 only)
WPK = 128 + 512 + 512 + 128 + 256  # 1536
# bpack bf16 cols: cis split (256) | cls_m (128) | clst (128)
BPK = H + HC + HC  # 512
# fpack bf16 cols: lpit rolled (512)
FPK = B

STW = 8  # stats cols: 0 d_red, 1 se, 2 scis, 3 sclip, 4 msp, 5 clst, 6 diag


def _build():
    nc = bass.Bass()

    wpack = nc.declare_dram_parameter("wpack", [P, WPK], F8, isOutput=False)
    fpack = nc.declare_dram_parameter("fpack", [P, FPK], BF16, isOutput=False)
    bpack = nc.declare_dram_parameter("bpack", [P, BPK], BF16, isOutput=False)
    out_p = nc.declare_dram_parameter("partials", [P, STW], F32, isOutput=True)

    ctx = ExitStack()

    def sb(shape, dtype, name):
        return ctx.enter_context(nc.sbuf_tensor(name, shape, dtype))

    def ps(shape, name):
        return ctx.enter_context(nc.psum_tensor(name, shape, F32))

    with ctx:
        ctx.enter_context(
            nc.allow_low_precision(reason="loss rel tolerance 2e-2; bf16 chain")
        )
        wpack_t = sb([P, WPK], F8, "wpack_t")
        bpack_t = sb([P, BPK], BF16, "bpack_t")
        fpack_t = sb([P, FPK], BF16, "fpack_t")
        ones_sb = sb([1, H], BF16, "ones_sb")
        urec = sb([P, H], BF16, "urec")
        sim_b = sb([P, H], BF16, "sim_b")
        csT = sb([P, H], BF16, "csT")
        nd = sb([P, H], BF16, "nd")
        e_b = sb([P, H], BF16, "e_b")
        prod = sb([P, H], BF16, "prod")
        ea_out = sb([P, H + HC], BF16, "ea_out")
        lnsp_out = sb([P, HC], BF16, "lnsp_out")
        eclip_out = sb([P, B], BF16, "eclip_out")
        stats = sb([P, STW], F32, "stats")

        psum_u = ps([P, H], "psum_u")
        psum_i = ps([P, H], "psum_i")

        # views
        def dr(apv):  # [128, 2k] -> [128, 2, k] DoubleRow planes
            return apv.rearrange("p (two f) -> p two f", two=2)

        comp_dr = dr(wpack_t[:, 0:128])
        wT_dr = [dr(wpack_t[:, 128:640]), dr(wpack_t[:, 640:1152])]
        ws_dr = dr(wpack_t[:, 1152:1280])

        def comp_k(k):  # [128, 64] plain chunk-k view of the DR pack
            return wpack_t[:, 64 * k : 64 * k + 64]

        def wT_k(k, h):  # [128, 256] chunk-k plane of wT_dr[h]
            c0 = 128 + 512 * h + 256 * k
            return wpack_t[:, c0 : c0 + 256]

        def ws_k(k):
            return wpack_t[:, 1152 + 64 * k : 1152 + 64 * k + 64]

        s_row = wpack_t[0:1, 1280:1536].bitcast(BF16)  # [1, 128]
        cis_v = bpack_t[:, 0:H]
        ecls_v = bpack_t[:, 0 : H + HC]
        clst_v = bpack_t[:, H + HC : BPK]
        esp_v = ea_out[:, H : H + HC]

        # ---------------- planner ----------------
        # per-column virtual handles so disjoint stats writers don't serialize
        class _Col:
            def __init__(self, j):
                self.j = j

        stats_cols = [_Col(j) for j in range(STW)]
        plan = []

        def op(eng, fn, reads, writes):
            plan.append((eng, fn, tuple(reads), tuple(writes)))

        V, A, T = "V", "A", "T"
        DR = mybir.MatmulPerfMode.DoubleRow

        op(V, lambda: nc.vector.memset(ones_sb[:, :], 1.0), [], [ones_sb])

        # --- PE: union first (DVE's urec hides the inter matmuls).
        # DoubleRow dst must start at partition 0 (s3d3_mm_valid_dst_partition),
        # so h0 halves use DoubleRow and h1 halves use plain per-chunk matmuls.
        op(T, lambda: nc.tensor.matmul(
            psum_u[0:R, :], comp_dr, wT_dr[0], start=True, stop=False,
            perf_mode=DR, skip_group_check=True), [wpack_t], [psum_u])
        op(T, lambda: nc.tensor.matmul(
            psum_u[R:P, :], comp_k(0), wT_k(0, 1), start=True, stop=False,
            skip_group_check=True), [wpack_t], [psum_u])
        op(T, lambda: nc.tensor.matmul(
            psum_u[R:P, :], comp_k(1), wT_k(1, 1), start=False, stop=False,
            skip_group_check=True), [wpack_t], [psum_u])
        op(T, lambda: nc.tensor.matmul(
            psum_u[:, :], s_row, ones_sb[:, :], start=False, stop=True,
            skip_group_check=True), [wpack_t, ones_sb], [psum_u])
        op(T, lambda: nc.tensor.matmul(
            psum_i[0:R, :], ws_dr, wT_dr[0], start=True, stop=True,
            perf_mode=DR, skip_group_check=True), [wpack_t], [psum_i])
        op(T, lambda: nc.tensor.matmul(
            psum_i[R:P, :], ws_k(0), wT_k(0, 1), start=True, stop=False,
            skip_group_check=True), [wpack_t], [psum_i])
        op(T, lambda: nc.tensor.matmul(
            psum_i[R:P, :], ws_k(1), wT_k(1, 1), start=False, stop=True,
            skip_group_check=True), [wpack_t], [psum_i])

        # --- DVE: Jaccard chain
        op(V, lambda: nc.vector.reciprocal(out=urec[:, :], in_=psum_u[:, :]),
           [psum_u], [urec])
        op(V, lambda: nc.vector.tensor_mul(sim_b[:, :], psum_i[:, :], urec[:, :]),
           [psum_i, urec], [sim_b])
        op(V, lambda: nc.vector.tensor_scalar(
            out=csT[:, :], in0=cis_v, scalar1=TEMP, scalar2=None, op0=ALU.mult),
           [bpack_t], [csT])
        op(V, lambda: nc.vector.tensor_sub(nd[:, :], csT[:, :], sim_b[:, :]),
           [csT, sim_b], [nd])

        # --- ACT: exps + ln; accums write stats directly
        op(A, lambda: nc.scalar.activation(out=ea_out[:, :], in_=ecls_v, func=AF.Exp),
           [bpack_t], [ea_out])
        op(A, lambda: nc.scalar.activation(
            out=lnsp_out[:, :], in_=esp_v, func=AF.Ln, bias=1.0,
            accum_out=stats[:, 4:5]), [ea_out], [lnsp_out, stats_cols[4]])
        op(A, lambda: nc.scalar.activation(
            out=e_b[:, :], in_=sim_b[:, :], func=AF.Exp, scale=1.0 / TEMP,
            accum_out=stats[:, 1:2]), [sim_b], [e_b, stats_cols[1]])
        op(A, lambda: nc.scalar.activation(
            out=eclip_out[:, :], in_=fpack_t[:, :], func=AF.Exp,
            accum_out=stats[:, 3:4]), [fpack_t], [eclip_out, stats_cols[3]])

        # --- DVE: reductions + tail
        op(V, lambda: nc.vector.reduce_sum(
            out=stats[:, 2:3], in_=ea_out[:, 0:H], axis=AX.X),
           [ea_out], [stats_cols[2]])
        op(V, lambda: nc.vector.reduce_sum(
            out=stats[:, 5:6], in_=clst_v, axis=AX.X), [bpack_t], [stats_cols[5]])
        op(V, lambda: nc.vector.tensor_mul(prod[:, :], e_b[:, :], nd[:, :]),
           [e_b, nd], [prod])
        op(V, lambda: nc.vector.reduce_sum(
            out=stats[:, 0:1], in_=prod[:, :], axis=AX.X), [prod], [stats_cols[0]])
        op(V, lambda: nc.vector.tensor_copy(
            out=stats[:, 6:7], in_=fpack_t[:, 0:1]), [fpack_t], [stats_cols[6]])

        # ---------------- two-pass emission ----------------
        # Cross-engine waits only: same-engine deps are program order.
        last_writer = {}
        dma_tiles = {"d_w": wpack_t, "d_f": fpack_t, "d_b": bpack_t}
        for name, tile_ in dma_tiles.items():
            last_writer[id(tile_)] = (name, 16)
        counts = {"V": 0, "A": 0, "T": 0}
        waits_needed = []
        for eng, fn, reads, writes in plan:
            need = {}
            for tset_i, tset in enumerate((reads, writes)):
                for tile_ in tset:
                    lw = last_writer.get(id(tile_))
                    assert tset_i == 1 or lw is not None, (
                        f"plan not topological: read of unwritten tile {tile_}"
                    )
                    if lw is not None:
                        k, t = lw
                        if k != eng and need.get(k, 0) < t:
                            need[k] = t
            waits_needed.append(sorted(need.items()))
            counts[eng] += 1
            for tile_ in writes:
                last_writer[id(tile_)] = (eng, counts[eng])
        stats_finals = {}
        cnt2 = {"V": 0, "A": 0, "T": 0}
        for eng, fn, reads, writes in plan:
            cnt2[eng] += 1
            for tile_ in writes:
                if tile_ in stats_cols:
                    stats_finals[eng] = cnt2[eng]

        with ExitStack() as semctx:
            sems = {}
            for k in ("V", "A", "T"):
                sems[k] = semctx.enter_context(nc.semaphore(f"sem_{k}"))
            for name in dma_tiles:
                sems[name] = semctx.enter_context(nc.semaphore(f"sem_{name}"))
            out_sem = semctx.enter_context(nc.semaphore("sem_out"))

            engines = {"V": nc.vector, "A": nc.scalar, "T": nc.tensor}
            observed = {k: {} for k in ("V", "A", "T")}

            def emit_for(eng):
                for (e, fn, reads, writes), need in zip(plan, waits_needed):
                    if e != eng:
                        continue
                    obs = observed[eng]
                    for k, t in need:
                        if obs.get(k, 0) < t:
                            engines[eng].wait_ge(sems[k], t)
                            obs[k] = t
                    instr = fn()
                    instr.then_inc(sems[eng], 1)

            with nc.Block(no_gpsimd_drain=True) as block:

                @block.sync
                def _(sync):
                    sync.dma_start(out=wpack_t[:], in_=wpack[:, :]).then_inc(
                        sems["d_w"], 16
                    )
                    sync.dma_start(out=fpack_t[:], in_=fpack[:, :]).then_inc(
                        sems["d_f"], 16
                    )
                    for eng_k, tick in sorted(stats_finals.items()):
                        sync.wait_ge(sems[eng_k], tick)
                    sync.dma_start(out=out_p[:, :], in_=stats[:, :]).then_inc(
                        out_sem, 16
                    )

                @block.gpsimd
                def _(gpsimd):
                    gpsimd.dma_start(out=bpack_t[:], in_=bpack[:, :]).then_inc(
                        sems["d_b"], 16
                    )

                @block.vector
                def _(vector):
                    emit_for("V")

                @block.scalar
                def _(scalar):
                    emit_for("A")

                @block.tensor
                def _(tensor):
                    emit_for("T")

    return nc


_NC = None


def _get_nc():
    global _NC
    if _NC is None:
        _NC = _build()
    return _NC


def _split(x):
    """[64, 2h] -> [128, h]: row i cols 0:h -> partition i; cols h:2h -> 64+i."""
    h = x.shape[1] // 2
    return np.concatenate([x[:, :h], x[:, h:]], axis=0)


def _dr_pack(m):
    """[256, k] -> [128, 2k]: channel planes side by side for DoubleRow."""
    return np.concatenate([m[0:P, :], m[P:C, :]], axis=1)


def make_in_maps(inputs):
    lpi = np.asarray(inputs["logits_per_image"], dtype=np.float32)
    lpt = np.asarray(inputs["logits_per_text"], dtype=np.float32)
    cl = np.asarray(inputs["concepts_logits"], dtype=np.float32)
    cis = np.asarray(inputs["concepts_image_similarity"], dtype=np.float32)
    mc = np.asarray(inputs["medical_concepts"], dtype=np.int32)

    w8 = (mc == 1).astype(np.int8)  # [B, C]
    w8T = w8.T  # [C, B]
    col = np.arange(B)[None, :]

    in_maps = []
    for i in range(M):
        r0 = i * R
        sl = slice(r0, r0 + R)
        rows = np.arange(R)[:, None]

        ws = w8[sl].T  # [C, R]
        comp = (1 - ws).astype(np.int8)
        s_row = w8[sl].sum(axis=1)  # [R] ints
        s128 = s_row[np.arange(P) % R].astype(BF16NP)  # [128]

        wpk = np.concatenate(
            [
                _dr_pack(comp).astype(F8NP).view(np.uint8),
                _dr_pack(w8T[:, 0:H]).astype(F8NP).view(np.uint8),
                _dr_pack(w8T[:, H:B]).astype(F8NP).view(np.uint8),
                _dr_pack(ws).astype(F8NP).view(np.uint8),
                np.broadcast_to(s128.view(np.uint8)[None, :], (P, 256)),
            ],
            axis=1,
        )  # [128, 1536] bytes

        # per-row roll so the label diagonal lands in column 0
        ridx = (col + (r0 + rows)) % B
        lpit = np.concatenate([lpi[sl][rows, ridx], lpt[sl][rows, ridx]], axis=0)
        fpk = lpit.astype(BF16NP)  # [128, 512]

        cls = cl[sl]
        mcs = mc[sl]
        cls_m = np.where(mcs == -1, -30.0, cls)
        clst = np.where(mcs == 1, cls, 0.0)
        bpk = np.concatenate(
            [_split(cis[sl]), _split(cls_m), _split(clst)], axis=1
        ).astype(BF16NP)  # [128, 512]

        in_maps.append(
            {
                "wpack": np.ascontiguousarray(wpk).view(F8NP),
                "fpack": np.ascontiguousarray(fpk),
                "bpack": np.ascontiguousarray(bpk),
            }
        )
    return in_maps


def combine_partials(per_core_partials, mask_count):
    lse_sum = 0.0
    diag_sum = 0.0
    msp_sum = 0.0
    q_sum = 0.0
    kl_sum = 0.0
    for p in per_core_partials:
        a = np.asarray(p, dtype=np.float64).reshape(P, STW)
        d_row = a[0:R, 0] + a[R:P, 0]  # device ships sum e*(T*cis - sim) = -d
        se_row = a[0:R, 1] + a[R:P, 1]
        sc_row = a[0:R, 2] + a[R:P, 2]
        kl_sum += np.sum(-d_row / (TEMP * se_row) - np.log(se_row) + np.log(sc_row))
        lse_sum += np.sum(np.log(a[:, 3]))
        diag_sum += np.sum(a[:, 6])
        msp_sum += np.sum(a[:, 4])
        q_sum += np.sum(a[:, 5])
    clip_loss = (lse_sum - diag_sum) / (2.0 * B)
    concept_loss = (msp_sum - q_sum) / (mask_count + 1e-8)
    concept_sim_loss = kl_sum / B
    total = (
        clip_loss
        + CONCEPT_WEIGHT * concept_loss
        + CONCEPT_SIM_WEIGHT * concept_sim_loss
    )
    return np.float32(total)


def run_spmd(inputs, **kwargs):
    in_maps = make_in_maps(inputs)
    return run_bass_kernel_spmd(_get_nc(), in_maps, core_ids=list(range(M)), **kwargs)


def kernel(**inputs):
    mc = np.asarray(inputs["medical_concepts"], dtype=np.int32)
    mask_count = float((mc != -1).sum())
    res = run_spmd(inputs)
    return combine_partials([r["partials"] for r in res.results], mask_count)
